# revision 22
# baseline (speedup 1.0000x reference)
"""MoE FFN (E=8 experts, top-2) — expert-parallel Bass/Tile kernel for 8 TRN2 cores.

Strategy:
  - Host computes the (tiny) router: logits = x @ gate_w.T, top-2 per token,
    renormalized weights (= sigmoid of logit differences).  This is the
    sharding decision: token n is dispatched to cores e1(n), e2(n).
  - Core e receives its expert's weights and the gathered, transposed
    tokens XgT [D, C], all in bf16.  C adapts to the actual max expert
    load (rounded up to 16), so no fixed over-capacity padding.
  - Device: per 512-token block: hT[hc] = gelu(w1.T @ xgT + b1)
    (feature-major), then the combine matmul is FLIPPED: stationary =
    w2 128x128 chunk, moving = hT tokens, so mm2 cost scales with real
    token count.  Output is yT [d, tokens], pre-gate.
  - All matmul operands are bf16 (error ~4e-3 << 2e-2 gate): weights are
    SBUF-resident (loaded once, 16.8 MB bf16), and bf16 stationaries get
    fast-weight-load so LDWEIGHTS hides under the matmul streams.
  - Host combine: out[idx_e] += g * Yg_e.T (each token appears in exactly
    2 experts), plus the gate-weighted b2 term.
"""

import re

import numpy as np
import ml_dtypes

import bass_rust
import concourse.bass as bass
import concourse.mybir as mybir
import concourse.tile as tile
from concourse import bacc, bass_utils

P = 128
D_MODEL = 1024
D_HID = 4096
E = 8
TOP_K = 2
N_CORES = 8

DC = D_MODEL // P          # 8 d-chunks (contraction for mm1)
HC = D_HID // P            # 32 h-chunks (contraction for mm2)
DQ = D_MODEL // P          # 8 output-d chunks (mm2 psum partitions)
HG = 4                     # h-chunks per w1 SBUF tile ([P, 512])
NHG = HC // HG             # 8 w1 tile groups
BLK = 512                  # token block (one fp32 PSUM bank wide)

F32 = mybir.dt.float32
BF16 = mybir.dt.bfloat16
NPBF16 = ml_dtypes.bfloat16

_tail_patched = False


def _patch_light_tail():
    """Replace Tile's end-of-context machinery (multi-wait drain + two
    all-engine EVSEM barriers + semaphore range-clears, ~10us on HW) with
    single-wait drains on the sync engine covering every logical proc's final
    tick.  The NEFF is executed once per load in this flow, so semaphores
    need not be recycled."""
    global _tail_patched
    if _tail_patched:
        return
    _tail_patched = True

    def _drain_and_barrier(self, tick_clock, wait_clock):
        gc = tick_clock.global_clock
        ticks = eval(re.match(r"VectorClock\((.*)\)", repr(gc)).group(1))
        n = len(ticks)
        for i, v in enumerate(ticks):
            if v > 0:
                vc = bass_rust.VectorClock(
                    [v if j == i else 0 for j in range(n)])
                w = self.nc.sync.drain()
                wait_clock.add_sem_waits(
                    w.ins,
                    bass_rust.ScopedClock({None: vc}),
                    bass_rust.ScopedClock({}),
                )
        popped = self.nc._tile_sem_poison_stack.pop()
        assert popped is self._sem_poison

    tile.TileContext._drain_and_barrier = _drain_and_barrier


def _blocks(C):
    # Equal-size blocks, each <= 512 (one fp32 PSUM bank) and as wide as
    # possible: a matmul's ~90ns LDWEIGHTS hides under the moving-operand
    # stream only when N is ~256+, so narrow remainder blocks (whose MMs
    # would be LDW-bound at ~5x the per-column cost) must not exist.
    # C is pre-padded by the caller so it divides evenly.
    nb = (C + BLK - 1) // BLK
    bt = C // nb
    assert bt * nb == C
    return [(i * bt, bt) for i in range(nb)]


def build_nc(C):
    _patch_light_tail()
    nc = bacc.Bacc("TRN2", target_bir_lowering=False, debug=False,
                   num_devices=N_CORES)

    # Inputs, pre-tiled on host into consumption order (all contiguous DMAs):
    #   xgt  [DC, P, C]          xgt[dc, p, n] = Xg[n, dc*128+p]
    #   w1t  [NHG, DC, P, HG*P]  w1t[hg, dc, p, k*128+j] = w1[dc*128+p, (hg*4+k)*128+j]
    #   w2t  [HC, P, D]          w2t[hc, p, d] = w2[hc*128+p, d]
    #   b1t  [P, HC]             b1t[p, hc] = b1[hc*128+p]
    #   ygt  [DQ, P, C]          ygt[dq, p, n] = y[n, dq*128+p]   (pre-gate)
    BLOCKS = _blocks(C)
    NB = len(BLOCKS)
    BT = BLOCKS[0][1]

    xgt = nc.dram_tensor("xgt", [DC, P, C], BF16, kind="ExternalInput")
    w1t = nc.dram_tensor("w1t", [NHG, DC, P, HG * P], BF16,
                         kind="ExternalInput")
    w2t = nc.dram_tensor("w2t", [HC, P, D_MODEL], BF16, kind="ExternalInput")
    b1t = nc.dram_tensor("b1t", [P, HC], F32, kind="ExternalInput")
    # ygt[bi, p, dq*BT + n] = y[bi*BT + n, dq*128 + p]  (pre-gate).  Block-
    # major wide layout so each output DMA moves [128 x DQ*BT/2] with long
    # contiguous rows (the per-token-strided layout ran at ~40 GB/s/queue
    # and dominated the kernel tail).
    ygt = nc.dram_tensor("ygt", [NB, P, DQ * BT], BF16, kind="ExternalOutput")

    with tile.TileContext(nc) as tc:
        with (
            tc.tile_pool(name="const", bufs=1) as const,
            tc.tile_pool(name="xg", bufs=1) as xg_pool,
            tc.tile_pool(name="w1", bufs=1) as w1_pool,
            tc.tile_pool(name="w2", bufs=1) as w2_pool,
            tc.tile_pool(name="ht", bufs=HC + 4) as ht_pool,
            tc.tile_pool(name="yo", bufs=2) as yo_pool,
            tc.tile_pool(name="ps1", bufs=4, space="PSUM") as ps1,
            tc.tile_pool(name="ps2", bufs=3, space="PSUM") as ps2,
        ):
            # Warm-up: ~10 junk matmuls on a zeroed tile, emitted first so
            # they run during the DMA-gated ramp (PE would idle there) and
            # flip the HAM clock gate to 8/8 before the real matmuls start.
            junk = const.tile([P, BLK], BF16, name="junk")
            nc.vector.memset(junk[:], 0)
            for i in range(10):
                ps = ps1.tile([P, BLK], F32, name="ps1")
                nc.tensor.matmul(ps[:], lhsT=junk[:, :P], rhs=junk[:],
                                 start=True, stop=True)

            # Head: the first mm1 chain consumes (w1[hg0,dc], xg[b0,dc]) in
            # dc order; interleave those pairs round-robin over all three
            # HWDGE rings so MM(dc) inputs land incrementally.
            rings = [nc.sync, nc.scalar, nc.gpsimd]
            w1_sb = {}
            xg_sb = {}
            k = 0
            bt0 = BLOCKS[0][1]
            n00 = BLOCKS[0][0]
            for dc in range(DC):
                t = w1_pool.tile([P, HG * P], BF16, name=f"w1_0_{dc}")
                rings[k % 3].dma_start(out=t[:], in_=w1t[0, dc, :, :])
                k += 1
                w1_sb[(0, dc)] = t
                t = xg_pool.tile([P, bt0], BF16, name=f"xg_0_{dc}")
                rings[k % 3].dma_start(out=t[:], in_=xgt[dc, :, n00:n00 + bt0])
                k += 1
                xg_sb[(0, dc)] = t
            b1_sb = const.tile([P, HC], F32, name="b1sb")
            nc.gpsimd.dma_start(out=b1_sb[:], in_=b1t[:, :])
            # scalar ring stops here (it runs the gelus); bulk on sync+gpsimd
            # in consumption order: w1 groups (needed from ~15us), then w2
            # (needed from ~50us), then the later xg blocks (needed ~95us+).
            bulk = [nc.sync, nc.gpsimd]
            for hg in range(1, NHG):
                for dc in range(DC):
                    t = w1_pool.tile([P, HG * P], BF16, name=f"w1_{hg}_{dc}")
                    bulk[k % 2].dma_start(out=t[:], in_=w1t[hg, dc, :, :])
                    k += 1
                    w1_sb[(hg, dc)] = t
            # first half of w2 goes over the scalar ring, interleaved with
            # block-0 gelus below (the two bulk rings alone deliver w1+w2
            # ~8us too late for mm2 of block 0); the rest follows w1 here.
            NSC = HC // 2
            w2_sb = [w2_pool.tile([P, D_MODEL], BF16, name=f"w2_{hc}")
                     for hc in range(HC)]
            for hc in range(NSC, HC):
                bulk[k % 2].dma_start(out=w2_sb[hc][:], in_=w2t[hc, :, :])
                k += 1
            for bi, (n0, BT) in enumerate(BLOCKS[1:], start=1):
                for dc in range(DC):
                    t = xg_pool.tile([P, BT], BF16, name=f"xg_{bi}_{dc}")
                    bulk[k % 2].dma_start(out=t[:], in_=xgt[dc, :, n0:n0 + BT])
                    k += 1
                    xg_sb[(bi, dc)] = t

            for bi, (n0, BT) in enumerate(BLOCKS):
                # ---- mm1: hT[hc] = gelu(w1.T @ xgT + b1) ----
                ht_tiles = []
                for hc in range(HC):
                    hg, kk = divmod(hc, HG)
                    ps = ps1.tile([P, BT], F32, name="ps1")
                    for dc in range(DC):
                        nc.tensor.matmul(
                            ps[:],
                            lhsT=w1_sb[(hg, dc)][:, kk * P:(kk + 1) * P],
                            rhs=xg_sb[(bi, dc)][:],
                            start=(dc == 0),
                            stop=(dc == DC - 1),
                        )
                    ht = ht_pool.tile([P, BLK], BF16, name="ht")
                    nc.scalar.activation(
                        ht[:, :BT], ps[:],
                        mybir.ActivationFunctionType.Gelu,
                        bias=b1_sb[:, hc:hc + 1],
                    )
                    ht_tiles.append(ht)
                    if bi == 0 and hc < NSC:
                        nc.scalar.dma_start(out=w2_sb[hc][:],
                                            in_=w2t[hc, :, :])

                # ---- mm2 (flipped): yT[dq] = sum_hc w2[hc,dq].T @ hT[hc] ----
                yo = yo_pool.tile([P, DQ * BT], BF16, name="yo")
                for dq in range(DQ):
                    ps = ps2.tile([P, BT], F32, name="ps2")
                    for hc in range(HC):
                        nc.tensor.matmul(
                            ps[:],
                            lhsT=w2_sb[hc][:, dq * P:(dq + 1) * P],
                            rhs=ht_tiles[hc][:, :BT],
                            start=(hc == 0),
                            stop=(hc == HC - 1),
                        )
                    nc.vector.tensor_scalar_mul(
                        yo[:, dq * BT:(dq + 1) * BT], ps[:], 1.0)
                    if dq % 2 == 1:
                        h0 = (dq - 1) * BT
                        rings[(4 * bi + dq // 2) % 3].dma_start(
                            out=ygt[bi, :, h0:(dq + 1) * BT],
                            in_=yo[:, h0:(dq + 1) * BT])
    nc.compile()
    return nc


_NC_CACHE = {}
TRACE = False
LAST_RESULTS = None


def _get_nc(C):
    if C not in _NC_CACHE:
        _NC_CACHE[C] = build_nc(C)
    return _NC_CACHE[C]


def kernel(x, gate_w, w1, b1, w2, b2):
    x = np.asarray(x, dtype=np.float32)
    gate_w = np.asarray(gate_w, dtype=np.float32)
    w1 = np.asarray(w1, dtype=np.float32)
    b1 = np.asarray(b1, dtype=np.float32)
    w2 = np.asarray(w2, dtype=np.float32)
    b2 = np.asarray(b2, dtype=np.float32)

    B, T, D = x.shape
    N = B * T
    xf = x.reshape(N, D)

    # ---- router (host; 0.05% of model FLOPs — this is the sharding step) ----
    logits = xf @ gate_w.T                           # [N, E]
    order = np.argsort(-logits, axis=1, kind="stable")
    i1, i2 = order[:, 0], order[:, 1]
    l1 = logits[np.arange(N), i1].astype(np.float64)
    l2 = logits[np.arange(N), i2].astype(np.float64)
    g1 = (1.0 / (1.0 + np.exp(l2 - l1))).astype(np.float32)
    g2 = (1.0 - g1).astype(np.float32)

    # ---- dispatch: gather per-expert tokens, pre-tile all inputs ----
    idx_per_e = []
    gv_per_e = []
    cnt_max = 0
    for e in range(E):
        sel1 = np.nonzero(i1 == e)[0]
        sel2 = np.nonzero(i2 == e)[0]
        idx = np.concatenate([sel1, sel2])
        gv = np.concatenate([g1[sel1], g2[sel2]])
        idx_per_e.append(idx)
        gv_per_e.append(gv)
        cnt_max = max(cnt_max, idx.shape[0])
    # C split into equal blocks of width <= BLK, each a multiple of 8
    C = max(cnt_max, P)
    nb = (C + BLK - 1) // BLK
    bt = (C + nb - 1) // nb
    bt = (bt + 1) // 2 * 2
    C = nb * bt

    in_maps = []
    for e in range(E):
        idx = idx_per_e[e]
        cnt = idx.shape[0]
        xg = np.zeros((C, D), NPBF16)
        xg[:cnt] = xf[idx]
        xgt = np.ascontiguousarray(xg.T.reshape(DC, P, C))
        w1t = np.ascontiguousarray(
            w1[e].astype(NPBF16).reshape(DC, P, NHG, HG * P)
            .transpose(2, 0, 1, 3))
        w2t = np.ascontiguousarray(
            w2[e].astype(NPBF16).reshape(HC, P, D_MODEL))
        b1t = np.ascontiguousarray(b1[e].reshape(HC, P).T)
        in_maps.append({"xgt": xgt, "w1t": w1t, "w2t": w2t, "b1t": b1t})

    nc = _get_nc(C)
    res = bass_utils.run_bass_kernel_spmd(
        nc, in_maps, core_ids=list(range(N_CORES)), trace=TRACE)
    global LAST_RESULTS
    LAST_RESULTS = res

    # ---- combine (host): each token occurs in exactly 2 experts, never twice
    # in one, so fancy-index += is safe per expert ----
    out = np.zeros((N, D), np.float32)
    for e in range(E):
        idx = idx_per_e[e]
        cnt = idx.shape[0]
        ygt = res.results[e]["ygt"]                  # [NB, P, DQ*BT] bf16
        # ygt[bi, p, dq*bt + n] = y[bi*bt + n, dq*128 + p]
        y = (ygt.reshape(nb, P, DQ, bt)
             .transpose(0, 3, 2, 1).reshape(C, D)[:cnt].astype(np.float32))
        out[idx] += gv_per_e[e][:, None] * y

    if np.any(b2):
        gate_full = np.zeros((N, E), np.float32)
        gate_full[np.arange(N), i1] = g1
        gate_full[np.arange(N), i2] = g2
        out += gate_full @ b2.reshape(E, D)

    return out.reshape(B, T, D)


# revision 25
# speedup vs baseline: 1.0508x; 1.0508x over previous
"""MoE FFN (E=8 experts, top-2) — expert-parallel Bass/Tile kernel for 8 TRN2 cores.

Strategy:
  - Host computes the (tiny) router: logits = x @ gate_w.T, top-2 per token,
    renormalized weights (= sigmoid of logit differences).  This is the
    sharding decision: token n is dispatched to cores e1(n), e2(n).
  - Core e receives its expert's weights and the gathered, transposed
    tokens XgT [D, C], all in bf16.  C adapts to the actual max expert
    load (rounded up to 16), so no fixed over-capacity padding.
  - Device: per 512-token block: hT[hc] = gelu(w1.T @ xgT + b1)
    (feature-major), then the combine matmul is FLIPPED: stationary =
    w2 128x128 chunk, moving = hT tokens, so mm2 cost scales with real
    token count.  Output is yT [d, tokens], pre-gate.
  - All matmul operands are bf16 (error ~4e-3 << 2e-2 gate): weights are
    SBUF-resident (loaded once, 16.8 MB bf16), and bf16 stationaries get
    fast-weight-load so LDWEIGHTS hides under the matmul streams.
  - Host combine: out[idx_e] += g * Yg_e.T (each token appears in exactly
    2 experts), plus the gate-weighted b2 term.
"""

import re

import numpy as np
import ml_dtypes

import bass_rust
import concourse.bass as bass
import concourse.mybir as mybir
import concourse.tile as tile
from concourse import bacc, bass_utils

P = 128
D_MODEL = 1024
D_HID = 4096
E = 8
TOP_K = 2
N_CORES = 8

DC = D_MODEL // P          # 8 d-chunks (contraction for mm1)
HC = D_HID // P            # 32 h-chunks (contraction for mm2)
DQ = D_MODEL // P          # 8 output-d chunks (mm2 psum partitions)
HG = 4                     # h-chunks per w1 SBUF tile ([P, 512])
NHG = HC // HG             # 8 w1 tile groups
BLK = 512                  # token block (one fp32 PSUM bank wide)

F32 = mybir.dt.float32
BF16 = mybir.dt.bfloat16
NPBF16 = ml_dtypes.bfloat16

_tail_patched = False


def _patch_light_tail():
    """Replace Tile's end-of-context machinery (multi-wait drain + two
    all-engine EVSEM barriers + semaphore range-clears, ~10us on HW) with
    single-wait drains on the sync engine covering every logical proc's final
    tick.  The NEFF is executed once per load in this flow, so semaphores
    need not be recycled."""
    global _tail_patched
    if _tail_patched:
        return
    _tail_patched = True

    def _drain_and_barrier(self, tick_clock, wait_clock):
        gc = tick_clock.global_clock
        ticks = eval(re.match(r"VectorClock\((.*)\)", repr(gc)).group(1))
        n = len(ticks)
        for i, v in enumerate(ticks):
            if v > 0:
                vc = bass_rust.VectorClock(
                    [v if j == i else 0 for j in range(n)])
                w = self.nc.sync.drain()
                wait_clock.add_sem_waits(
                    w.ins,
                    bass_rust.ScopedClock({None: vc}),
                    bass_rust.ScopedClock({}),
                )
        popped = self.nc._tile_sem_poison_stack.pop()
        assert popped is self._sem_poison

    tile.TileContext._drain_and_barrier = _drain_and_barrier


def _blocks(C):
    # Equal-size blocks, each <= 512 (one fp32 PSUM bank) and as wide as
    # possible: a matmul's ~90ns LDWEIGHTS hides under the moving-operand
    # stream only when N is ~256+, so narrow remainder blocks (whose MMs
    # would be LDW-bound at ~5x the per-column cost) must not exist.
    # C is pre-padded by the caller so it divides evenly.
    nb = (C + BLK - 1) // BLK
    bt = C // nb
    assert bt * nb == C
    return [(i * bt, bt) for i in range(nb)]


def build_nc(C):
    _patch_light_tail()
    nc = bacc.Bacc("TRN2", target_bir_lowering=False, debug=False,
                   num_devices=N_CORES)

    # Inputs, pre-tiled on host into consumption order (all contiguous DMAs):
    #   xgt  [DC, P, C]          xgt[dc, p, n] = Xg[n, dc*128+p]
    #   w1t  [NHG, DC, P, HG*P]  w1t[hg, dc, p, k*128+j] = w1[dc*128+p, (hg*4+k)*128+j]
    #   w2t  [HC, P, D]          w2t[hc, p, d] = w2[hc*128+p, d]
    #   b1t  [P, HC]             b1t[p, hc] = b1[hc*128+p]
    #   ygt  [DQ, P, C]          ygt[dq, p, n] = y[n, dq*128+p]   (pre-gate)
    BLOCKS = _blocks(C)
    NB = len(BLOCKS)
    BT = BLOCKS[0][1]

    xgt = nc.dram_tensor("xgt", [DC, P, C], BF16, kind="ExternalInput")
    w1t = nc.dram_tensor("w1t", [NHG, DC, P, HG * P], BF16,
                         kind="ExternalInput")
    w2t = nc.dram_tensor("w2t", [HC, P, D_MODEL], BF16, kind="ExternalInput")
    b1t = nc.dram_tensor("b1t", [P, HC], F32, kind="ExternalInput")
    # ygt[bi, p, dq*BT + n] = y[bi*BT + n, dq*128 + p]  (pre-gate).  Block-
    # major wide layout so each output DMA moves [128 x DQ*BT/2] with long
    # contiguous rows (the per-token-strided layout ran at ~40 GB/s/queue
    # and dominated the kernel tail).
    ygt = nc.dram_tensor("ygt", [NB, P, DQ * BT], BF16, kind="ExternalOutput")

    with tile.TileContext(nc) as tc:
        with (
            tc.tile_pool(name="const", bufs=1) as const,
            tc.tile_pool(name="xg", bufs=1) as xg_pool,
            tc.tile_pool(name="w1", bufs=1) as w1_pool,
            tc.tile_pool(name="w2", bufs=1) as w2_pool,
            tc.tile_pool(name="ht", bufs=HC + 4) as ht_pool,
            tc.tile_pool(name="yo", bufs=2) as yo_pool,
            tc.tile_pool(name="ps1", bufs=5, space="PSUM") as ps1,
            tc.tile_pool(name="ps2", bufs=3, space="PSUM") as ps2,
        ):
            # Warm-up: ~10 junk matmuls on a zeroed tile, emitted first so
            # they run during the DMA-gated ramp (PE would idle there) and
            # flip the HAM clock gate to 8/8 before the real matmuls start.
            junk = const.tile([P, BLK], BF16, name="junk")
            nc.vector.memset(junk[:], 0)
            for i in range(10):
                ps = ps1.tile([P, BLK], F32, name="ps1")
                nc.tensor.matmul(ps[:], lhsT=junk[:, :P], rhs=junk[:],
                                 start=True, stop=True)

            # Head: the first mm1 chain consumes (w1[hg0,dc], xg[b0,dc]) in
            # dc order; interleave those pairs round-robin over all three
            # HWDGE rings so MM(dc) inputs land incrementally.
            rings = [nc.sync, nc.scalar, nc.gpsimd]
            w1_sb = {}
            xg_sb = {}
            k = 0
            bt0 = BLOCKS[0][1]
            n00 = BLOCKS[0][0]
            for dc in range(DC):
                t = w1_pool.tile([P, HG * P], BF16, name=f"w1_0_{dc}")
                rings[k % 3].dma_start(out=t[:], in_=w1t[0, dc, :, :])
                k += 1
                w1_sb[(0, dc)] = t
                t = xg_pool.tile([P, bt0], BF16, name=f"xg_0_{dc}")
                rings[k % 3].dma_start(out=t[:], in_=xgt[dc, :, n00:n00 + bt0])
                k += 1
                xg_sb[(0, dc)] = t
            b1_sb = const.tile([P, HC], F32, name="b1sb")
            nc.gpsimd.dma_start(out=b1_sb[:], in_=b1t[:, :])
            w2_sb = [w2_pool.tile([P, D_MODEL], BF16, name=f"w2_{hc}")
                     for hc in range(HC)]
            # w2[0:8] rides the scalar ring's early slack (the two bulk
            # rings alone deliver w1+w2 ~8us too late for mm2 of block 0);
            # enqueued before any gelu so it doesn't slow gelu pacing.
            NSC = 8
            for hc in range(NSC):
                nc.scalar.dma_start(out=w2_sb[hc][:], in_=w2t[hc, :, :])
            # bulk on sync+gpsimd in consumption order: w1 groups (needed
            # from ~15us), then w2 (needed from ~50us), then the later xg
            # blocks (needed ~95us+).
            bulk = [nc.sync, nc.gpsimd]
            for hg in range(1, NHG):
                for dc in range(DC):
                    t = w1_pool.tile([P, HG * P], BF16, name=f"w1_{hg}_{dc}")
                    bulk[k % 2].dma_start(out=t[:], in_=w1t[hg, dc, :, :])
                    k += 1
                    w1_sb[(hg, dc)] = t
            for hc in range(NSC, HC):
                bulk[k % 2].dma_start(out=w2_sb[hc][:], in_=w2t[hc, :, :])
                k += 1
            for bi, (n0, BT) in enumerate(BLOCKS[1:], start=1):
                for dc in range(DC):
                    t = xg_pool.tile([P, BT], BF16, name=f"xg_{bi}_{dc}")
                    bulk[k % 2].dma_start(out=t[:], in_=xgt[dc, :, n0:n0 + BT])
                    k += 1
                    xg_sb[(bi, dc)] = t

            for bi, (n0, BT) in enumerate(BLOCKS):
                # ---- mm1: hT[hc] = gelu(w1.T @ xgT + b1) ----
                ht_tiles = []
                for hc in range(HC):
                    hg, kk = divmod(hc, HG)
                    ps = ps1.tile([P, BT], F32, name="ps1")
                    for dc in range(DC):
                        nc.tensor.matmul(
                            ps[:],
                            lhsT=w1_sb[(hg, dc)][:, kk * P:(kk + 1) * P],
                            rhs=xg_sb[(bi, dc)][:],
                            start=(dc == 0),
                            stop=(dc == DC - 1),
                        )
                    ht = ht_pool.tile([P, BLK], BF16, name="ht")
                    nc.scalar.activation(
                        ht[:, :BT], ps[:],
                        mybir.ActivationFunctionType.Gelu,
                        bias=b1_sb[:, hc:hc + 1],
                    )
                    ht_tiles.append(ht)

                # ---- mm2 (flipped): yT[dq] = sum_hc w2[hc,dq].T @ hT[hc] ----
                yo = yo_pool.tile([P, DQ * BT], BF16, name="yo")
                for dq in range(DQ):
                    ps = ps2.tile([P, BT], F32, name="ps2")
                    for hc in range(HC):
                        nc.tensor.matmul(
                            ps[:],
                            lhsT=w2_sb[hc][:, dq * P:(dq + 1) * P],
                            rhs=ht_tiles[hc][:, :BT],
                            start=(hc == 0),
                            stop=(hc == HC - 1),
                        )
                    nc.vector.tensor_scalar_mul(
                        yo[:, dq * BT:(dq + 1) * BT], ps[:], 1.0)
                    if dq % 2 == 1:
                        h0 = (dq - 1) * BT
                        rings[(4 * bi + dq // 2) % 3].dma_start(
                            out=ygt[bi, :, h0:(dq + 1) * BT],
                            in_=yo[:, h0:(dq + 1) * BT])
    nc.compile()
    return nc


_NC_CACHE = {}
TRACE = False
LAST_RESULTS = None


def _get_nc(C):
    if C not in _NC_CACHE:
        _NC_CACHE[C] = build_nc(C)
    return _NC_CACHE[C]


def kernel(x, gate_w, w1, b1, w2, b2):
    x = np.asarray(x, dtype=np.float32)
    gate_w = np.asarray(gate_w, dtype=np.float32)
    w1 = np.asarray(w1, dtype=np.float32)
    b1 = np.asarray(b1, dtype=np.float32)
    w2 = np.asarray(w2, dtype=np.float32)
    b2 = np.asarray(b2, dtype=np.float32)

    B, T, D = x.shape
    N = B * T
    xf = x.reshape(N, D)

    # ---- router (host; 0.05% of model FLOPs — this is the sharding step) ----
    logits = xf @ gate_w.T                           # [N, E]
    order = np.argsort(-logits, axis=1, kind="stable")
    i1, i2 = order[:, 0], order[:, 1]
    l1 = logits[np.arange(N), i1].astype(np.float64)
    l2 = logits[np.arange(N), i2].astype(np.float64)
    g1 = (1.0 / (1.0 + np.exp(l2 - l1))).astype(np.float32)
    g2 = (1.0 - g1).astype(np.float32)

    # ---- dispatch: gather per-expert tokens, pre-tile all inputs ----
    idx_per_e = []
    gv_per_e = []
    cnt_max = 0
    for e in range(E):
        sel1 = np.nonzero(i1 == e)[0]
        sel2 = np.nonzero(i2 == e)[0]
        idx = np.concatenate([sel1, sel2])
        gv = np.concatenate([g1[sel1], g2[sel2]])
        idx_per_e.append(idx)
        gv_per_e.append(gv)
        cnt_max = max(cnt_max, idx.shape[0])
    # C split into equal blocks of width <= BLK, each a multiple of 8
    C = max(cnt_max, P)
    nb = (C + BLK - 1) // BLK
    bt = (C + nb - 1) // nb
    bt = (bt + 1) // 2 * 2
    C = nb * bt

    in_maps = []
    for e in range(E):
        idx = idx_per_e[e]
        cnt = idx.shape[0]
        xg = np.zeros((C, D), NPBF16)
        xg[:cnt] = xf[idx]
        xgt = np.ascontiguousarray(xg.T.reshape(DC, P, C))
        w1t = np.ascontiguousarray(
            w1[e].astype(NPBF16).reshape(DC, P, NHG, HG * P)
            .transpose(2, 0, 1, 3))
        w2t = np.ascontiguousarray(
            w2[e].astype(NPBF16).reshape(HC, P, D_MODEL))
        b1t = np.ascontiguousarray(b1[e].reshape(HC, P).T)
        in_maps.append({"xgt": xgt, "w1t": w1t, "w2t": w2t, "b1t": b1t})

    nc = _get_nc(C)
    res = bass_utils.run_bass_kernel_spmd(
        nc, in_maps, core_ids=list(range(N_CORES)), trace=TRACE)
    global LAST_RESULTS
    LAST_RESULTS = res

    # ---- combine (host): each token occurs in exactly 2 experts, never twice
    # in one, so fancy-index += is safe per expert ----
    out = np.zeros((N, D), np.float32)
    for e in range(E):
        idx = idx_per_e[e]
        cnt = idx.shape[0]
        ygt = res.results[e]["ygt"]                  # [NB, P, DQ*BT] bf16
        # ygt[bi, p, dq*bt + n] = y[bi*bt + n, dq*128 + p]
        y = (ygt.reshape(nb, P, DQ, bt)
             .transpose(0, 3, 2, 1).reshape(C, D)[:cnt].astype(np.float32))
        out[idx] += gv_per_e[e][:, None] * y

    if np.any(b2):
        gate_full = np.zeros((N, E), np.float32)
        gate_full[np.arange(N), i1] = g1
        gate_full[np.arange(N), i2] = g2
        out += gate_full @ b2.reshape(E, D)

    return out.reshape(B, T, D)
